# revision 1
# baseline (speedup 1.0000x reference)
"""Trainium2 Bass kernel for nn_CausalAttention_84018150244353.

kernel(**inputs) takes the FULL unsharded inputs (as in reference
setup_inputs) and returns the full (2, 2048, 2048) float32 output.

Sharding: 8 NeuronCores = 2 batches x 4 head-groups (4 heads each).
Each core computes its head-group's QKV projections, causal ALiBi
attention (transposed layout, fp32r matmuls), and its partial output
projection y^T = Wo_s^T @ out^T; the host sums the 4 head-group
partials per batch and adds bo.

See build_program() docstring for the per-core algorithm.
"""
import math
import os
import sys
import time

sys.path.insert(0, "/opt/trn_rl_repo")

import numpy as np
import jax

jax.config.update("jax_compilation_cache_dir",
                  os.environ.get("JAX_NEFF_CACHE", "/tmp/jax_neff_cache"))
jax.config.update("jax_persistent_cache_min_compile_time_secs", 0.0)
jax.config.update("jax_persistent_cache_min_entry_size_bytes", -1)

from jax.sharding import Mesh, PartitionSpec
from jax.experimental.shard_map import shard_map

import concourse.bass as bass
import concourse.mybir as mybir
import concourse.tile as tile
from concourse import bacc
from concourse import bass2jax
from concourse.bass2jax import _bass_exec_p, install_neuronx_cc_hook

f32 = mybir.dt.float32
r32 = mybir.dt.float32r
Exp = mybir.ActivationFunctionType.Exp

T = 2048
EMB = 2048
HG = 512          # columns per head group (4 heads x 128)
HD = 128
NH = 4            # heads per core
NQ = 4            # T quarters
QT = T // NQ      # 512
NE = EMB // 128   # 16 contraction chunks
NC_I = 4          # i-chunks of 512 queries
NJ = T // 128     # 16 key chunks


def build_program(reps: int = 1):
    nc = bacc.Bacc("TRN2", target_bir_lowering=False, debug=False,
                   enable_asserts=False, num_devices=8)

    xT_d = nc.dram_tensor("xT", [EMB, T], r32, kind="ExternalInput")
    wq_d = nc.dram_tensor("wq", [EMB, HG], r32, kind="ExternalInput")
    wk_d = nc.dram_tensor("wk", [EMB, HG], r32, kind="ExternalInput")
    wv_d = nc.dram_tensor("wv", [EMB, HG], r32, kind="ExternalInput")
    wo_d = nc.dram_tensor("wo", [HG, T], r32, kind="ExternalInput")
    bmat_d = nc.dram_tensor("bmat", [128, 2, HG], r32, kind="ExternalInput")
    bv_d = nc.dram_tensor("bv", [1, HG], r32, kind="ExternalInput")
    alibi_d = nc.dram_tensor("alibi", [128, NH * NJ], f32, kind="ExternalInput")
    mshift_d = nc.dram_tensor("mshift", [1, NH * T], r32, kind="ExternalInput")
    mask_d = nc.dram_tensor("maskadd", [128, 4, 512], r32, kind="ExternalInput")
    ident_d = nc.dram_tensor("ident", [128, 128], r32, kind="ExternalInput")
    onesrow_d = nc.dram_tensor("onesrow", [1, 512], r32, kind="ExternalInput")
    yT_d = nc.dram_tensor("yT", [T, T], f32, kind="ExternalOutput")

    with tile.TileContext(nc) as tc:
        with (
            tc.tile_pool(name="consts", bufs=1) as consts,
            tc.tile_pool(name="dramp", bufs=1, space="DRAM") as dramp,
        ):
            qT_t = [dramp.tile([HG, QT], r32, name=f"qT_t{q}") for q in range(NQ)]
            kT_t = [dramp.tile([HG, QT], r32, name=f"kT_t{q}") for q in range(NQ)]
            v_t = [dramp.tile([QT, HG], r32, name=f"v_t{q}") for q in range(NQ)]

            def body():
                # ---- constants (loaded once, first) ----
                onesb_sb = consts.tile([128, 512], r32, name="onesb_sb")
                alibi_sb = consts.tile([128, NH * NJ], f32, name="alibi_sb")
                mask_sb = consts.tile([128, 4, 512], r32, name="mask_sb")
                ident_sb = consts.tile([128, 128], r32, name="ident_sb")
                nc.sync.dma_start(
                    onesb_sb[:], onesrow_d.ap().to_broadcast((128, 512)))
                nc.sync.dma_start(alibi_sb[:], alibi_d.ap())
                nc.sync.dma_start(mask_sb[:], mask_d.ap())
                nc.sync.dma_start(ident_sb[:], ident_d.ap())

                # ================= Phase 1: projections =================
                with (
                    tc.tile_pool(name="wslab", bufs=1) as wslab,
                    tc.tile_pool(name="xslab", bufs=2) as xslab,
                    tc.tile_pool(name="p1ps", bufs=4, space="PSUM") as p1ps,
                    tc.tile_pool(name="p1st", bufs=4) as p1st,
                ):
                    xT_r = xT_d.ap().rearrange("(c p) t -> p c t", p=128)

                    # first x quarter before the weight slabs: unblocks the
                    # first projection block as early as possible
                    x_sb = xslab.tile([128, NE, QT], r32, name="x_sb",
                                      tag="x_sb")
                    nc.sync.dma_start(x_sb[:], xT_r[:, :, 0:QT])

                    bmat_sb = wslab.tile([128, 2, HG], r32, name="bmat_sb")
                    bvb_sb = wslab.tile([128, HG], r32, name="bvb_sb")
                    nc.sync.dma_start(bmat_sb[:], bmat_d.ap())
                    nc.sync.dma_start(
                        bvb_sb[:], bv_d.ap().to_broadcast((128, HG)))
                    wq_sb = wslab.tile([128, NE, HG], r32, name="wq_sb")
                    wk_sb = wslab.tile([128, NE, HG], r32, name="wk_sb")
                    wv_sb = wslab.tile([128, NE, HG], r32, name="wv_sb")
                    wq_r = wq_d.ap().rearrange("(c p) m -> p c m", p=128)
                    wk_r = wk_d.ap().rearrange("(c p) m -> p c m", p=128)
                    wv_r = wv_d.ap().rearrange("(c p) m -> p c m", p=128)
                    for cc in range(4):
                        nc.sync.dma_start(
                            wq_sb[:, :, cc * 128:(cc + 1) * 128],
                            wq_r[:, :, cc * 128:(cc + 1) * 128])
                    for cc in range(4):
                        nc.sync.dma_start(
                            wk_sb[:, :, cc * 128:(cc + 1) * 128],
                            wk_r[:, :, cc * 128:(cc + 1) * 128])
                    nc.sync.dma_start(wv_sb[:], wv_r)

                    for qt in range(NQ):
                        if qt > 0:
                            x_sb = xslab.tile([128, NE, QT], r32, name="x_sb",
                                              tag="x_sb")
                            nc.sync.dma_start(
                                x_sb[:], xT_r[:, :, qt * QT:(qt + 1) * QT])

                        for pi, (w_sb, dst) in enumerate(
                                ((wq_sb, qT_t), (wk_sb, kT_t))):
                            for cc in range(4):
                                ps = p1ps.tile([128, 512], f32, name="p1acc",
                                               tag="p1acc")
                                for e in range(NE):
                                    nc.tensor.matmul(
                                        ps[:],
                                        w_sb[:, e, cc * 128:(cc + 1) * 128],
                                        x_sb[:, e, :],
                                        start=(e == 0), stop=False)
                                nc.tensor.matmul(
                                    ps[:],
                                    bmat_sb[:, pi, cc * 128:(cc + 1) * 128],
                                    onesb_sb[:],
                                    start=False, stop=True)
                                st = p1st.tile([128, 512], r32, name="p1out",
                                               tag="p1out")
                                nc.scalar.copy(st[:], ps[:])
                                nc.sync.dma_start(
                                    dst[qt][cc * 128:(cc + 1) * 128, :],
                                    st[:])

                        for tb in range(4):
                            ps = p1ps.tile([128, 512], f32, name="p1acc",
                                           tag="p1acc")
                            for e in range(NE):
                                nc.tensor.matmul(
                                    ps[:],
                                    x_sb[:, e, tb * 128:(tb + 1) * 128],
                                    wv_sb[:, e, :],
                                    start=(e == 0), stop=False)
                            nc.tensor.matmul(
                                ps[:],
                                onesb_sb[:, 0:128],
                                bvb_sb[:],
                                start=False, stop=True)
                            st = p1st.tile([128, 512], r32, name="p1out",
                                           tag="p1out")
                            nc.scalar.copy(st[:], ps[:])
                            nc.sync.dma_start(
                                v_t[qt][tb * 128:(tb + 1) * 128, :], st[:])

                # ========= Phase 2+3: attention + inlined projection =========
                with (
                    tc.tile_pool(name="qkv", bufs=1) as qkv,
                    tc.tile_pool(name="wop", bufs=1) as wop,
                    tc.tile_pool(name="outfp", bufs=2) as outfp,
                    tc.tile_pool(name="pp", bufs=4) as pp,
                    tc.tile_pool(name="smallp", bufs=2) as smallp,
                    tc.tile_pool(name="mshp", bufs=3) as mshp,
                    tc.tile_pool(name="p3st", bufs=3) as p3st,
                    tc.tile_pool(name="ps_s", bufs=4, space="PSUM") as ps_s,
                    tc.tile_pool(name="ps_o", bufs=2, space="PSUM") as ps_o,
                    tc.tile_pool(name="ps_d", bufs=2, space="PSUM") as ps_d,
                ):
                    # wo early: no deps, loads during attention ramp-up
                    wo_sb = wop.tile([128, NH, T], r32, name="wo_sb")
                    nc.sync.dma_start(
                        wo_sb[:],
                        wo_d.ap().rearrange("(h p) o -> p h o", p=128))

                    qT_sbq, kT_sbq, v_sbq = [], [], []
                    for q in range(NQ):
                        qq = qkv.tile([128, NH, QT], r32, name=f"qT_sb{q}")
                        kq = qkv.tile([128, NH, QT], r32, name=f"kT_sb{q}")
                        vq = qkv.tile([128, 4, HG], r32, name=f"v_sb{q}")
                        nc.sync.dma_start(
                            qq[:], qT_t[q][:].rearrange("(h p) t -> p h t",
                                                        p=128))
                        nc.sync.dma_start(
                            kq[:], kT_t[q][:].rearrange("(h p) t -> p h t",
                                                        p=128))
                        nc.sync.dma_start(
                            vq[:], v_t[q][:].rearrange("(c p) m -> p c m",
                                                       p=128))
                        qT_sbq.append(qq); kT_sbq.append(kq); v_sbq.append(vq)

                    LOOK = 3
                    for c in range(NC_I):
                        outf_c = outfp.tile([128, NH, 512], r32,
                                            name="outf_c", tag="outf_c")
                        for h in range(NH):
                            msh_sb = mshp.tile([128, 512], r32, name="msh_sb",
                                               tag="msh_sb")
                            nc.sync.dma_start(
                                msh_sb[:],
                                mshift_d.ap()[0:1, h * T + c * 512:
                                              h * T + (c + 1) * 512]
                                .to_broadcast((128, 512)))
                            nj = 4 * c + 4

                            s_tiles = {}
                            p_tiles = {}

                            def emit_scores(jc):
                                s = ps_s.tile([128, 512], f32, name="s_ps",
                                              tag="s_ps")
                                diag = jc >= 4 * c
                                nc.tensor.matmul(
                                    s[:],
                                    kT_sbq[jc // 4][:, h,
                                                    (jc % 4) * 128:
                                                    (jc % 4 + 1) * 128],
                                    qT_sbq[c][:, h, :],
                                    start=True, stop=False)
                                nc.tensor.matmul(
                                    s[:], ident_sb[:], msh_sb[:],
                                    start=False, stop=not diag)
                                if diag:
                                    nc.tensor.matmul(
                                        s[:], ident_sb[:],
                                        mask_sb[:, jc - 4 * c, :],
                                        start=False, stop=True)
                                s_tiles[jc] = s

                            def emit_exp(jc):
                                p = pp.tile([128, 512], r32, name="p_sb",
                                            tag="p_sb")
                                nc.scalar.activation(
                                    p[:], s_tiles.pop(jc)[:], Exp,
                                    bias=alibi_sb[:, h * NJ + jc:
                                                  h * NJ + jc + 1])
                                p_tiles[jc] = p

                            outp = ps_o.tile([128, 512], f32, name="out_ps",
                                             tag="out_ps")
                            den = ps_d.tile([128, 512], f32, name="den_ps",
                                            tag="den_ps")

                            def emit_consume(jc):
                                p = p_tiles.pop(jc)
                                nc.tensor.matmul(
                                    outp[:],
                                    v_sbq[jc // 4][:, jc % 4,
                                                   h * 128:(h + 1) * 128],
                                    p[:],
                                    start=(jc == 0), stop=(jc == nj - 1))
                                nc.tensor.matmul(
                                    den[:], onesb_sb[:, 0:128], p[:],
                                    start=(jc == 0), stop=(jc == nj - 1))

                            for jc in range(min(LOOK, nj)):
                                emit_scores(jc)
                            for jc in range(nj):
                                if jc + LOOK < nj:
                                    emit_scores(jc + LOOK)
                                emit_exp(jc)
                                emit_consume(jc)

                            rcp = smallp.tile([128, 512], f32, name="rcp",
                                              tag="rcp")
                            with nc.allow_low_precision(
                                    reason="elementwise reciprocal"):
                                nc.vector.reciprocal(rcp[:], den[:])
                            nc.vector.tensor_mul(
                                outf_c[:, h, :], outp[:], rcp[:])

                        # ---- inlined output projection for this i-chunk ----
                        for oc in range(16):
                            yp = ps_s.tile([128, 512], f32, name="y_ps",
                                           tag="s_ps")
                            for h in range(NH):
                                nc.tensor.matmul(
                                    yp[:],
                                    wo_sb[:, h, oc * 128:(oc + 1) * 128],
                                    outf_c[:, h, :],
                                    start=(h == 0), stop=(h == 3))
                            ys = p3st.tile([128, 512], f32, name="y_sb",
                                           tag="y_sb")
                            nc.scalar.copy(ys[:], yp[:])
                            nc.sync.dma_start(
                                yT_d.ap()[oc * 128:(oc + 1) * 128,
                                          c * 512:(c + 1) * 512],
                                ys[:])

            if reps == 1:
                body()
            else:
                with tc.For_i(0, reps, 1):
                    body()

    nc.compile()
    return nc


def make_host_inputs(x, Wq, bq, Wk, bk, Wv, bv, Wo, bo):
    """Shard full inputs into 8 per-core input maps."""
    import math
    import numpy as np

    x = np.asarray(x, np.float32)
    Wq = np.asarray(Wq, np.float32); bq = np.asarray(bq, np.float32)
    Wk = np.asarray(Wk, np.float32); bk = np.asarray(bk, np.float32)
    Wv = np.asarray(Wv, np.float32); bv = np.asarray(bv, np.float32)
    Wo = np.asarray(Wo, np.float32)

    NUM_HEAD = 16
    start = 2 ** (-2 ** (-(math.log2(NUM_HEAD) - 3)))
    slopes = np.array([start * start ** i for i in range(NUM_HEAD)], np.float32)

    sc = np.float32(1.0 / math.sqrt(HD))
    jl = np.arange(128, dtype=np.float32)
    jcs = np.arange(NJ, dtype=np.float32)
    key_idx = (jcs[None, :] * 128 + jl[:, None])  # [128, NJ]

    il = np.arange(512, dtype=np.float32)
    mm = np.arange(4, dtype=np.float32)
    cond = (128 * mm[None, :, None] + jl[:, None, None]) > il[None, None, :]
    maskadd = np.where(cond, np.float32(-1e10), np.float32(0.0))

    ident = np.eye(128, dtype=np.float32)
    onesrow = np.ones((1, 512), np.float32)
    i_idx = np.arange(T, dtype=np.float32)

    in_maps = []
    for core in range(8):
        b, hg = core // 4, core % 4
        cols = slice(hg * HG, (hg + 1) * HG)
        heads = slopes[hg * NH:(hg + 1) * NH]
        alibi = np.empty((128, NH * NJ), np.float32)
        mshift = np.empty((1, NH * T), np.float32)
        for h in range(NH):
            alibi[:, h * NJ:(h + 1) * NJ] = -heads[h] * (T - 1 - key_idx)
            mshift[0, h * T:(h + 1) * T] = heads[h] * (T - 1 - i_idx)
        bmat = np.zeros((128, 2, HG), np.float32)
        bmat[0, 0, :] = bq[cols] * sc
        bmat[0, 1, :] = bk[cols]
        in_maps.append({
            "xT": np.ascontiguousarray(x[b].T),
            "wq": np.ascontiguousarray(Wq[:, cols]) * sc,
            "wk": np.ascontiguousarray(Wk[:, cols]),
            "wv": np.ascontiguousarray(Wv[:, cols]),
            "wo": np.ascontiguousarray(Wo[cols, :]),
            "bmat": bmat,
            "bv": bv[cols].reshape(1, HG),
            "alibi": alibi,
            "mshift": mshift,
            "maskadd": maskadd,
            "ident": ident,
            "onesrow": onesrow,
        })
    return in_maps


def assemble_output(results, bo):
    """results: list of 8 per-core dicts with 'yT'. Returns (2, T, EMB)."""
    import numpy as np
    bo = np.asarray(bo, np.float32)
    out = np.empty((2, T, EMB), np.float32)
    for b in range(2):
        acc = results[b * 4 + 0]["yT"].copy()
        for hg in range(1, 4):
            acc += results[b * 4 + hg]["yT"]
        out[b] = acc.T + bo
    return out


class SpmdRunner:
    def __init__(self, nc, n_cores: int):
        install_neuronx_cc_hook()
        self.nc = nc
        self.n_cores = n_cores
        assert nc.dbg_addr is None or not nc.dbg_callbacks
        partition_name = (
            nc.partition_id_tensor.name if nc.partition_id_tensor else None
        )
        in_names, out_names, out_avals = [], [], []
        for alloc in nc.m.functions[0].allocations:
            if not isinstance(alloc, mybir.MemoryLocationSet):
                continue
            name = alloc.memorylocations[0].name
            if alloc.kind == "ExternalInput":
                if name != partition_name:
                    in_names.append(name)
            elif alloc.kind == "ExternalOutput":
                shape = tuple(alloc.tensor_shape)
                dtype = mybir.dt.np(alloc.dtype)
                out_names.append(name)
                out_avals.append(jax.core.ShapedArray(shape, dtype))
        self.in_names = list(in_names)
        self.out_names = out_names
        self.out_avals = out_avals
        n_params = len(self.in_names)
        all_in_names = list(in_names) + list(out_names)
        if partition_name is not None:
            all_in_names.append(partition_name)
        self.partition_name = partition_name

        def _body(*args):
            operands = list(args)
            if partition_name is not None:
                operands.append(bass2jax.partition_id_tensor())
            outs = _bass_exec_p.bind(
                *operands,
                out_avals=tuple(out_avals),
                in_names=tuple(all_in_names),
                out_names=tuple(out_names),
                lowering_input_output_aliases=(),
                sim_require_finite=True,
                sim_require_nnan=True,
                nc=nc,
            )
            return tuple(outs)

        devices = jax.devices()[:n_cores]
        assert len(devices) == n_cores
        self.mesh = Mesh(np.asarray(devices), ("core",))
        n_outs = len(out_names)
        in_specs = (PartitionSpec("core"),) * (n_params + n_outs)
        out_specs = (PartitionSpec("core"),) * n_outs
        self.fn = jax.jit(
            shard_map(_body, mesh=self.mesh, in_specs=in_specs,
                      out_specs=out_specs, check_rep=False),
            keep_unused=True,
        )
        self.dev_args = None

    def set_inputs(self, in_maps: list[dict]):
        """device_put concatenated per-core inputs + zero output buffers."""
        n = self.n_cores
        assert len(in_maps) == n
        concat_in = [
            np.concatenate([np.asarray(in_maps[c][name]) for c in range(n)], axis=0)
            for name in self.in_names
        ]
        concat_zeros = [
            np.zeros((n * a.shape[0], *a.shape[1:]), a.dtype) for a in self.out_avals
        ]
        sharding = jax.sharding.NamedSharding(self.mesh, PartitionSpec("core"))
        self.dev_args = [jax.device_put(a, sharding) for a in concat_in + concat_zeros]

    def run(self):
        outs = self.fn(*self.dev_args)
        jax.block_until_ready(outs)
        return outs

    def results(self, outs) -> list[dict]:
        n = self.n_cores
        return [
            {
                name: np.asarray(outs[i]).reshape(n, *self.out_avals[i].shape)[c]
                for i, name in enumerate(self.out_names)
            }
            for c in range(n)
        ]

    def time_execs(self, iters: int = 10, warmup: int = 2):
        for _ in range(warmup):
            self.run()
        t0 = time.perf_counter()
        for _ in range(iters):
            outs = self.fn(*self.dev_args)
        jax.block_until_ready(outs)
        t1 = time.perf_counter()
        return (t1 - t0) / iters


_RUNNER = None


def _get_runner():
    global _RUNNER
    if _RUNNER is None:
        nc = build_program(reps=1)
        _RUNNER = SpmdRunner(nc, 8)
    return _RUNNER


def kernel(x, Wq, bq, Wk, bk, Wv, bv, Wo, bo):
    r = _get_runner()
    in_maps = make_host_inputs(x, Wq, bq, Wk, bk, Wv, bv, Wo, bo)
    r.set_inputs(in_maps)
    outs = r.run()
    res = r.results(outs)
    return assemble_output(res, bo)



# revision 2
# speedup vs baseline: 1.6844x; 1.6844x over previous
"""Trainium2 Bass kernel for nn_CausalAttention_84018150244353.

kernel(**inputs) takes the FULL unsharded inputs (as in reference
setup_inputs) and returns the full (2, 2048, 2048) float32 output.

Sharding: 8 NeuronCores = 2 batches x 4 head-sets. Core (b, m) owns
global heads {m, m+4, m+8, m+12} (interleaved so every core's local
head 0 has the largest slope of its set, local heads 1-3 small slopes).

Per-core pipeline (all matmul operands bf16, PSUM f32):
  Phase 1: QKV projections, outputs kept SBUF-resident in transposed
    layouts (qT/kT: [hd, head, t], v: [key, chunk, col]). Q/K biases
    (and the 1/sqrt(hd) scale, folded into Wq host-side) are applied
    via Identity-activation bias on the PSUM->SBUF copy.
  Phase 2: causal ALiBi attention per (query-chunk c, head h) in the
    transposed score layout s^T = [keys 128, queries <=512]. The ALiBi
    bias + causal shift are applied inside the Exp activation via
    per-partition (per-key) bias vectors: exponent' = s + slope*(j -
    i_ref) with i_ref a per-query-group reference (group=128 queries
    for local head 0, 512 for heads 1-3). The residual per-query
    factor e^{slope*(i - i_ref)} multiplies numerator and denominator
    identically and cancels in the softmax division. Diagonal 128x128
    blocks are masked by a tri01 elementwise multiply; diagonal tiles
    only compute the unmasked query span. Denominator via ones-matmul.
  Phase 3 (inlined per c): partial output projection y^T += Wo_h^T @
    outf_h; host sums the 4 head-set partials per batch and adds bo.
"""
import math
import os
import sys
import time

sys.path.insert(0, "/opt/trn_rl_repo")

import numpy as np
import jax

jax.config.update("jax_compilation_cache_dir",
                  os.environ.get("JAX_NEFF_CACHE", "/tmp/jax_neff_cache"))
jax.config.update("jax_persistent_cache_min_compile_time_secs", 0.0)
jax.config.update("jax_persistent_cache_min_entry_size_bytes", -1)

from jax.sharding import Mesh, PartitionSpec
from jax.experimental.shard_map import shard_map

import concourse.bass as bass
import concourse.mybir as mybir
import concourse.tile as tile
from concourse import bacc
from concourse import bass2jax
from concourse.bass2jax import _bass_exec_p, install_neuronx_cc_hook

f32 = mybir.dt.float32
bf16 = mybir.dt.bfloat16
Exp = mybir.ActivationFunctionType.Exp
Ident = mybir.ActivationFunctionType.Identity

T = 2048
EMB = 2048
HG = 512          # columns per head set (4 heads x 128)
HD = 128
NH = 4            # heads per core
NQ = 4            # T quarters
QT = T // NQ      # 512
NE = EMB // 128   # 16 contraction chunks
NJ = T // 128     # 16 key chunks
NEG = -1.0e30


def build_program(reps: int = 1):
    nc = bacc.Bacc("TRN2", target_bir_lowering=False, debug=False,
                   enable_asserts=False, num_devices=8)

    xT_d = nc.dram_tensor("xT", [EMB, T], bf16, kind="ExternalInput")
    wq_d = nc.dram_tensor("wq", [EMB, HG], bf16, kind="ExternalInput")
    wk_d = nc.dram_tensor("wk", [EMB, HG], bf16, kind="ExternalInput")
    wv_d = nc.dram_tensor("wv", [EMB, HG], bf16, kind="ExternalInput")
    wo_d = nc.dram_tensor("wo", [HG, T], bf16, kind="ExternalInput")
    bqv_d = nc.dram_tensor("bqv", [128, NH], f32, kind="ExternalInput")
    bkv_d = nc.dram_tensor("bkv", [128, NH], f32, kind="ExternalInput")
    bv_d = nc.dram_tensor("bv", [1, HG], bf16, kind="ExternalInput")
    biasT_d = nc.dram_tensor("biasT", [128, 64], f32, kind="ExternalInput")
    tri_d = nc.dram_tensor("tri", [128, 128], bf16, kind="ExternalInput")
    ones_d = nc.dram_tensor("ones", [128, 128], bf16, kind="ExternalInput")
    yT_d = nc.dram_tensor("yT", [T, T], f32, kind="ExternalOutput")

    with tile.TileContext(nc) as tc:
        with (
            tc.tile_pool(name="consts", bufs=1) as consts,
            tc.tile_pool(name="qkv", bufs=1) as qkv,
        ):
            def body():
                # ---- constants ----
                biasT_sb = consts.tile([128, 64], f32, name="biasT_sb")
                tri_sb = consts.tile([128, 128], bf16, name="tri_sb")
                ones_sb = consts.tile([128, 128], bf16, name="ones_sb")
                bqv_sb = consts.tile([128, NH], f32, name="bqv_sb")
                bkv_sb = consts.tile([128, NH], f32, name="bkv_sb")
                bv_sb = consts.tile([1, HG], bf16, name="bv_sb")
                nc.sync.dma_start(biasT_sb[:], biasT_d.ap())
                nc.sync.dma_start(tri_sb[:], tri_d.ap())
                nc.sync.dma_start(ones_sb[:], ones_d.ap())
                nc.sync.dma_start(bqv_sb[:], bqv_d.ap())
                nc.sync.dma_start(bkv_sb[:], bkv_d.ap())
                nc.sync.dma_start(bv_sb[:], bv_d.ap())

                # ---- persistent bf16 SBUF tensors ----
                qT_sb = qkv.tile([128, NH, T], bf16, name="qT_sb")
                kT_sb = qkv.tile([128, NH, T], bf16, name="kT_sb")
                v_sb = qkv.tile([128, NJ, HG], bf16, name="v_sb")
                outf_sb = qkv.tile([128, NH, T], bf16, name="outf_sb")

                # ================= Phase 1: projections =================
                with (
                    tc.tile_pool(name="wslab", bufs=1) as wslab,
                    tc.tile_pool(name="xslab", bufs=2) as xslab,
                    tc.tile_pool(name="p1ps", bufs=4, space="PSUM") as p1ps,
                ):
                    xT_r = xT_d.ap().rearrange("(c p) t -> p c t", p=128)

                    # first x quarter before the weight slabs
                    x_sb = xslab.tile([128, NE, QT], bf16, name="x_sb",
                                      tag="x_sb")
                    nc.sync.dma_start(x_sb[:], xT_r[:, :, 0:QT])

                    wq_sb = wslab.tile([128, NE, HG], bf16, name="wq_sb")
                    wk_sb = wslab.tile([128, NE, HG], bf16, name="wk_sb")
                    wv_sb = wslab.tile([128, NE, HG], bf16, name="wv_sb")
                    wq_r = wq_d.ap().rearrange("(c p) m -> p c m", p=128)
                    wk_r = wk_d.ap().rearrange("(c p) m -> p c m", p=128)
                    wv_r = wv_d.ap().rearrange("(c p) m -> p c m", p=128)
                    for cc in range(4):
                        nc.sync.dma_start(
                            wq_sb[:, :, cc * 128:(cc + 1) * 128],
                            wq_r[:, :, cc * 128:(cc + 1) * 128])
                    for cc in range(4):
                        nc.sync.dma_start(
                            wk_sb[:, :, cc * 128:(cc + 1) * 128],
                            wk_r[:, :, cc * 128:(cc + 1) * 128])
                    nc.sync.dma_start(wv_sb[:], wv_r)

                    for qt in range(NQ):
                        if qt > 0:
                            x_sb = xslab.tile([128, NE, QT], bf16,
                                              name="x_sb", tag="x_sb")
                            nc.sync.dma_start(
                                x_sb[:], xT_r[:, :, qt * QT:(qt + 1) * QT])

                        # Q^T and K^T: [outcol 128 (== head), t 512]
                        for w_sb, dstT, bvec in ((wq_sb, qT_sb, bqv_sb),
                                                 (wk_sb, kT_sb, bkv_sb)):
                            for h in range(NH):
                                ps = p1ps.tile([128, QT], f32, name="p1acc",
                                               tag="p1acc")
                                for e in range(NE):
                                    nc.tensor.matmul(
                                        ps[:],
                                        w_sb[:, e, h * 128:(h + 1) * 128],
                                        x_sb[:, e, :],
                                        start=(e == 0), stop=(e == NE - 1))
                                nc.scalar.activation(
                                    dstT[:, h, qt * QT:(qt + 1) * QT], ps[:],
                                    Ident, bias=bvec[:, h:h + 1])

                        # V: [t 128, col 512]
                        for tb in range(4):
                            ps = p1ps.tile([128, QT], f32, name="p1acc",
                                           tag="p1acc")
                            for e in range(NE):
                                nc.tensor.matmul(
                                    ps[:],
                                    x_sb[:, e, tb * 128:(tb + 1) * 128],
                                    wv_sb[:, e, :],
                                    start=(e == 0), stop=False)
                            nc.tensor.matmul(
                                ps[:], ones_sb[0:1, :], bv_sb[:],
                                start=False, stop=True)
                            nc.scalar.copy(v_sb[:, 4 * qt + tb, :], ps[:])

                # ========= Phase 2+3: attention + inlined projection =========
                with (
                    tc.tile_pool(name="wop", bufs=1) as wop,
                    tc.tile_pool(name="pp", bufs=5) as pp,
                    tc.tile_pool(name="rcpp", bufs=2) as rcpp,
                    tc.tile_pool(name="ystp", bufs=3) as ystp,
                    tc.tile_pool(name="ps_s", bufs=4, space="PSUM") as ps_s,
                    tc.tile_pool(name="ps_o", bufs=2, space="PSUM") as ps_o,
                    tc.tile_pool(name="ps_d", bufs=2, space="PSUM") as ps_d,
                ):
                    # wo early: no deps, loads during attention ramp-up
                    wo_sb = wop.tile([128, NH, T], bf16, name="wo_sb")
                    nc.sync.dma_start(
                        wo_sb[:],
                        wo_d.ap().rearrange("(h p) o -> p h o", p=128))

                    LOOK = 3
                    for c in range(NQ):
                        for h in range(NH):
                            nj = 4 * c + 4
                            s_tiles = {}
                            p_tiles = {}

                            def q0_of(jc):
                                g_rel = jc - 4 * c
                                return 128 * g_rel if g_rel > 0 else 0

                            def emit_scores(jc):
                                q0 = q0_of(jc)
                                s = ps_s.tile([128, 512], f32, name="s_ps",
                                              tag="s_ps")
                                nc.tensor.matmul(
                                    s[:, q0:512],
                                    kT_sb[:, h, jc * 128:(jc + 1) * 128],
                                    qT_sb[:, h, c * 512 + q0:(c + 1) * 512],
                                    start=True, stop=True)
                                s_tiles[jc] = s

                            def emit_exp(jc):
                                q0 = q0_of(jc)
                                g_rel = jc - 4 * c
                                s = s_tiles.pop(jc)
                                p = pp.tile([128, 512], bf16, name="p_sb",
                                            tag="p_sb")
                                if h == 0:
                                    for g in range(q0 // 128, 4):
                                        d = jc - 4 * c - g  # in [-15, 0]
                                        nc.scalar.activation(
                                            p[:, g * 128:(g + 1) * 128],
                                            s[:, g * 128:(g + 1) * 128],
                                            Exp,
                                            bias=biasT_sb[:, 15 + d:16 + d])
                                else:
                                    col = 16 + 16 * (h - 1) + (g_rel + 12)
                                    nc.scalar.activation(
                                        p[:, q0:512], s[:, q0:512], Exp,
                                        bias=biasT_sb[:, col:col + 1])
                                if g_rel >= 0:
                                    nc.vector.tensor_mul(
                                        p[:, q0:q0 + 128],
                                        p[:, q0:q0 + 128], tri_sb[:])
                                p_tiles[jc] = p

                            outp = ps_o.tile([128, 512], f32, name="out_ps",
                                             tag="out_ps")
                            den = ps_d.tile([128, 512], f32, name="den_ps",
                                            tag="den_ps")

                            def emit_consume(jc):
                                q0 = q0_of(jc)
                                p = p_tiles.pop(jc)
                                nc.tensor.matmul(
                                    outp[:, q0:512],
                                    v_sb[:, jc, h * 128:(h + 1) * 128],
                                    p[:, q0:512],
                                    start=(jc == 0), stop=(jc == nj - 1))
                                nc.tensor.matmul(
                                    den[:, q0:512], ones_sb[:], p[:, q0:512],
                                    start=(jc == 0), stop=(jc == nj - 1))

                            for jc in range(min(LOOK, nj)):
                                emit_scores(jc)
                            for jc in range(nj):
                                if jc + LOOK < nj:
                                    emit_scores(jc + LOOK)
                                emit_exp(jc)
                                emit_consume(jc)

                            rcp = rcpp.tile([128, 512], f32, name="rcp",
                                            tag="rcp")
                            with nc.allow_low_precision(
                                    reason="elementwise reciprocal"):
                                nc.vector.reciprocal(rcp[:], den[:])
                            nc.vector.tensor_mul(
                                outf_sb[:, h, c * 512:(c + 1) * 512],
                                outp[:], rcp[:])

                        # ---- inlined output projection for this i-chunk ----
                        for oc in range(16):
                            yp = ps_s.tile([128, 512], f32, name="y_ps",
                                           tag="s_ps")
                            for h in range(NH):
                                nc.tensor.matmul(
                                    yp[:],
                                    wo_sb[:, h, oc * 128:(oc + 1) * 128],
                                    outf_sb[:, h, c * 512:(c + 1) * 512],
                                    start=(h == 0), stop=(h == 3))
                            ys = ystp.tile([128, 512], f32, name="y_sb",
                                           tag="y_sb")
                            nc.scalar.copy(ys[:], yp[:])
                            nc.sync.dma_start(
                                yT_d.ap()[oc * 128:(oc + 1) * 128,
                                          c * 512:(c + 1) * 512],
                                ys[:])

            if reps == 1:
                body()
            else:
                with tc.For_i(0, reps, 1):
                    body()

    nc.compile()
    return nc


def make_host_inputs(x, Wq, bq, Wk, bk, Wv, bv, Wo, bo):
    """Shard full inputs into 8 per-core input maps."""
    import ml_dtypes
    tobf = lambda a: np.ascontiguousarray(a).astype(ml_dtypes.bfloat16)

    x = np.asarray(x, np.float32)
    Wq = np.asarray(Wq, np.float32); bq = np.asarray(bq, np.float32)
    Wk = np.asarray(Wk, np.float32); bk = np.asarray(bk, np.float32)
    Wv = np.asarray(Wv, np.float32); bv = np.asarray(bv, np.float32)
    Wo = np.asarray(Wo, np.float32)

    NUM_HEAD = 16
    start = 2 ** (-2 ** (-(math.log2(NUM_HEAD) - 3)))
    slopes = np.array([start * start ** i for i in range(NUM_HEAD)],
                      np.float32)
    sc = np.float32(1.0 / math.sqrt(HD))
    jl = np.arange(128, dtype=np.float32)

    tri01 = (jl[:, None] <= jl[None, :]).astype(np.float32)  # keys x queries
    ones128 = np.ones((128, 128), np.float32)

    in_maps = []
    for core in range(8):
        b, m = core // 4, core % 4
        heads = [m + 4 * hh for hh in range(NH)]      # interleaved
        cols = np.concatenate(
            [np.arange(g * HD, (g + 1) * HD) for g in heads])
        sl = slopes[heads]                             # local slopes, desc.

        biasT = np.zeros((128, 64), np.float32)
        for col in range(16):                          # local head 0
            d = col - 15
            biasT[:, col] = sl[0] * (128.0 * d + jl - 63.5)
        for hh in range(1, NH):                        # local heads 1-3
            for t in range(16):
                d4 = t - 12
                biasT[:, 16 + 16 * (hh - 1) + t] = (
                    sl[hh] * (128.0 * d4 + jl - 255.5))

        xTb = x[b].T                                   # [EMB, T]
        in_maps.append({
            "xT": tobf(xTb),
            "wq": tobf(Wq[:, cols] * sc),
            "wk": tobf(Wk[:, cols]),
            "wv": tobf(Wv[:, cols]),
            "wo": tobf(Wo[cols, :]),
            "bqv": np.ascontiguousarray(
                (bq[cols] * sc).reshape(NH, 128).T).astype(np.float32),
            "bkv": np.ascontiguousarray(
                bk[cols].reshape(NH, 128).T).astype(np.float32),
            "bv": tobf(bv[cols].reshape(1, HG)),
            "biasT": biasT,
            "tri": tobf(tri01),
            "ones": tobf(ones128),
        })
    return in_maps


def assemble_output(results, bo):
    """results: list of 8 per-core dicts with 'yT'. Returns (2, T, EMB)."""
    bo = np.asarray(bo, np.float32)
    out = np.empty((2, T, EMB), np.float32)
    for b in range(2):
        acc = results[b * 4 + 0]["yT"].copy()
        for hg in range(1, 4):
            acc += results[b * 4 + hg]["yT"]
        out[b] = acc.T + bo
    return out


class SpmdRunner:
    def __init__(self, nc, n_cores: int):
        install_neuronx_cc_hook()
        self.nc = nc
        self.n_cores = n_cores
        assert nc.dbg_addr is None or not nc.dbg_callbacks
        partition_name = (
            nc.partition_id_tensor.name if nc.partition_id_tensor else None
        )
        in_names, out_names, out_avals = [], [], []
        for alloc in nc.m.functions[0].allocations:
            if not isinstance(alloc, mybir.MemoryLocationSet):
                continue
            name = alloc.memorylocations[0].name
            if alloc.kind == "ExternalInput":
                if name != partition_name:
                    in_names.append(name)
            elif alloc.kind == "ExternalOutput":
                shape = tuple(alloc.tensor_shape)
                dtype = mybir.dt.np(alloc.dtype)
                out_names.append(name)
                out_avals.append(jax.core.ShapedArray(shape, dtype))
        self.in_names = list(in_names)
        self.out_names = out_names
        self.out_avals = out_avals
        n_params = len(self.in_names)
        all_in_names = list(in_names) + list(out_names)
        if partition_name is not None:
            all_in_names.append(partition_name)
        self.partition_name = partition_name

        def _body(*args):
            operands = list(args)
            if partition_name is not None:
                operands.append(bass2jax.partition_id_tensor())
            outs = _bass_exec_p.bind(
                *operands,
                out_avals=tuple(out_avals),
                in_names=tuple(all_in_names),
                out_names=tuple(out_names),
                lowering_input_output_aliases=(),
                sim_require_finite=True,
                sim_require_nnan=True,
                nc=nc,
            )
            return tuple(outs)

        devices = jax.devices()[:n_cores]
        assert len(devices) == n_cores
        self.mesh = Mesh(np.asarray(devices), ("core",))
        n_outs = len(out_names)
        in_specs = (PartitionSpec("core"),) * (n_params + n_outs)
        out_specs = (PartitionSpec("core"),) * n_outs
        self.fn = jax.jit(
            shard_map(_body, mesh=self.mesh, in_specs=in_specs,
                      out_specs=out_specs, check_rep=False),
            keep_unused=True,
        )
        self.dev_args = None

    def set_inputs(self, in_maps: list[dict]):
        """device_put concatenated per-core inputs + zero output buffers."""
        n = self.n_cores
        assert len(in_maps) == n
        concat_in = [
            np.concatenate([np.asarray(in_maps[c][name]) for c in range(n)],
                           axis=0)
            for name in self.in_names
        ]
        concat_zeros = [
            np.zeros((n * a.shape[0], *a.shape[1:]), a.dtype)
            for a in self.out_avals
        ]
        sharding = jax.sharding.NamedSharding(self.mesh, PartitionSpec("core"))
        self.dev_args = [jax.device_put(a, sharding)
                         for a in concat_in + concat_zeros]

    def run(self):
        outs = self.fn(*self.dev_args)
        jax.block_until_ready(outs)
        return outs

    def results(self, outs) -> list[dict]:
        n = self.n_cores
        return [
            {
                name: np.asarray(outs[i]).reshape(
                    n, *self.out_avals[i].shape)[c]
                for i, name in enumerate(self.out_names)
            }
            for c in range(n)
        ]

    def time_execs(self, iters: int = 10, warmup: int = 2):
        for _ in range(warmup):
            self.run()
        t0 = time.perf_counter()
        for _ in range(iters):
            outs = self.fn(*self.dev_args)
        jax.block_until_ready(outs)
        t1 = time.perf_counter()
        return (t1 - t0) / iters


_RUNNER = None


def _get_runner():
    global _RUNNER
    if _RUNNER is None:
        nc = build_program(reps=1)
        _RUNNER = SpmdRunner(nc, 8)
    return _RUNNER


def kernel(x, Wq, bq, Wk, bk, Wv, bv, Wo, bo):
    r = _get_runner()
    in_maps = make_host_inputs(x, Wq, bq, Wk, bk, Wv, bv, Wo, bo)
    r.set_inputs(in_maps)
    outs = r.run()
    res = r.results(outs)
    return assemble_output(res, bo)
